# revision 6
# baseline (speedup 1.0000x reference)
"""Trainium2 Bass kernel for nn_AttentionBlock (ragged_sequence, 16 equal
segments of 2048 q/kv tokens, HID=256, QD=64) on 8 NeuronCores.

Sharding: 2 segments (4096 rows) per core, weights replicated, outputs
concatenated host-side (attention is block-diagonal per segment -> no
cross-core communication needed).

v2: software-pipelined scores/exp vs att/epilogue, fp8 P+V with DoubleRow
att matmuls, wide exp slices, gpsimd epilogue offload.
"""

import os
import sys

os.environ.setdefault("MYCRO_LOCAL_CACHE", "1")
if "/opt/trn_rl_repo" not in sys.path:
    sys.path.insert(0, "/opt/trn_rl_repo")

import numpy as np

HID = 256
QD = 64
LQ = 2048
LH = 2048
B = 16
NCORES = 8
SEGS = 2                  # segments per core
ROWS = SEGS * LQ          # 4096 q rows per core
EPS = 1e-5
SCALE = 1.0 / 8.0         # 1/sqrt(QD)
NJT = LH // 128           # 16 key tiles per segment
NJP = NJT // 2            # 8 key-tile pairs per segment
NIC = 2                   # 1024-col query chunks per segment
ICW = LQ // NIC           # 1024
NIL = ICW // 128          # 8 query row-tiles per chunk
VST = 272                 # fp8 V block stride (256 V + 1 ones + pad, 16-aligned)

_built = {}


def _patch_act_tables():
    """Make the act-table pass choose the combined exp+ln table for every
    activation: blank all other tables (indices preserved so walrus's
    act_func_set_id remap stays correct). Avoids 100+ ACT_TABLE_LOADs
    (1.28us each) from alternating Exp/Ln table picks."""
    import functools
    import concourse.hw_specs as hw_specs
    import concourse.bacc as bacc_mod
    if getattr(hw_specs, "_attn_tables_patched", False):
        return
    orig = hw_specs.get_activation_tables

    @functools.cache
    def patched(arch):
        tabs = dict(orig(arch))
        joint = "natural_log_exp_and_others"
        assert joint in tabs, sorted(tabs)
        return {name: (funcs if name == joint else set())
                for name, funcs in tabs.items()}

    hw_specs.get_activation_tables = patched
    bacc_mod.get_activation_tables = patched
    hw_specs._attn_tables_patched = True


def _build(apply0: bool):
    from concourse import bacc, bass, mybir, tile

    _patch_act_tables()

    dt = mybir.dt
    f32 = dt.float32
    bf16 = dt.bfloat16
    f8 = dt.float8e4
    AF = mybir.ActivationFunctionType
    Alu = mybir.AluOpType
    DR = mybir.MatmulPerfMode.DoubleRow

    nc = bacc.Bacc("TRN2", target_bir_lowering=False, debug=False,
                   enable_asserts=False)

    qT_d = nc.dram_tensor("qT", [HID, ROWS], bf16, kind="ExternalInput")
    q_d = nc.dram_tensor("q", [ROWS, HID], f32, kind="ExternalInput")
    hT_d = nc.dram_tensor("hT", [HID, ROWS], bf16, kind="ExternalInput")
    wqT_d = nc.dram_tensor("WQT", [HID, QD], bf16, kind="ExternalInput")
    wkT_d = nc.dram_tensor("WKT", [HID, QD], bf16, kind="ExternalInput")
    wvT_d = nc.dram_tensor("WVT", [HID, HID], bf16, kind="ExternalInput")
    fwT_d = nc.dram_tensor("FCWT", [HID, HID], bf16, kind="ExternalInput")
    fb_d = nc.dram_tensor("FCB", [1, HID], bf16, kind="ExternalInput")
    idt_d = nc.dram_tensor("IDT", [128, 128], bf16, kind="ExternalInput")
    if apply0:
        n0w_d = nc.dram_tensor("N0W", [128, HID], f32, kind="ExternalInput")
        n0b_d = nc.dram_tensor("N0B", [128, HID], f32, kind="ExternalInput")
    out_d = nc.dram_tensor("out", [ROWS, HID], f32, kind="ExternalOutput")

    qT_a, q_a, hT_a = qT_d.ap(), q_d.ap(), hT_d.ap()
    out_a = out_d.ap()

    with tile.TileContext(nc) as tc:
        with (
            tc.tile_pool(name="const", bufs=1) as cpool,
            tc.tile_pool(name="kqq", bufs=1) as kqq_pool,
            tc.tile_pool(name="vsb", bufs=1) as v_pool,
            tc.tile_pool(name="qhT", bufs=1) as qh_pool,
            tc.tile_pool(name="pt", bufs=18) as pt_pool,
            tc.tile_pool(name="qrow", bufs=18) as q_pool,
            tc.tile_pool(name="ep", bufs=4) as ep_pool,
            tc.tile_pool(name="ep8", bufs=18) as ep8_pool,
            tc.tile_pool(name="st8", bufs=8) as st8_pool,
            tc.tile_pool(name="outp", bufs=6) as o_pool,
            tc.tile_pool(name="ps_st", bufs=1,
                         space=bass.MemorySpace.PSUM) as ps_st,
        ):
            # ---- constants ----
            wq_sb = cpool.tile([128, 2 * QD], bf16)     # [e, (chunk, c)]
            wk_sb = cpool.tile([128, 2 * QD], bf16)
            wv_sb = cpool.tile([128, 2 * HID], bf16)    # [e, (chunk, d)]
            fw_sb = cpool.tile([128, 2 * HID], bf16)    # fc_w.T chunks
            fb_sb = cpool.tile([1, HID], bf16)
            one_sb = cpool.tile([1, 128], bf16)
            idt_sb = cpool.tile([128, 128], bf16)
            for e in range(2):
                nc.sync.dma_start(wq_sb[:, e * QD:(e + 1) * QD],
                                  wqT_d.ap()[e * 128:(e + 1) * 128, :])
                nc.sync.dma_start(wk_sb[:, e * QD:(e + 1) * QD],
                                  wkT_d.ap()[e * 128:(e + 1) * 128, :])
                nc.sync.dma_start(wv_sb[:, e * HID:(e + 1) * HID],
                                  wvT_d.ap()[e * 128:(e + 1) * 128, :])
                nc.sync.dma_start(fw_sb[:, e * HID:(e + 1) * HID],
                                  fwT_d.ap()[e * 128:(e + 1) * 128, :])
            nc.sync.dma_start(fb_sb[:], fb_d.ap()[:, :])
            nc.sync.dma_start(idt_sb[:], idt_d.ap()[:, :])
            nc.vector.memset(one_sb[:], 1.0)
            eps_sb = cpool.tile([128, 1], f32)
            nc.vector.memset(eps_sb[:], EPS)
            nb3_sb = cpool.tile([128, 1], f32)
            nc.vector.memset(nb3_sb[:], -3.0)
            if apply0:
                n0w_sb = cpool.tile([128, HID], f32)
                n0b_sb = cpool.tile([128, HID], f32)
                nc.sync.dma_start(n0w_sb[:], n0w_d.ap()[:, :])
                nc.sync.dma_start(n0b_sb[:], n0b_d.ap()[:, :])

            # persistent activations
            # kT2: row-tiled K^T. partitions 0-63: even key tiles, 64-127:
            # odd key tiles; per seg 8 pair-blocks of 128 cols.
            kT2_sb = kqq_pool.tile([128, SEGS * LH // 2], bf16)
            # qq2: qq^T duplicated into both partition halves.
            qq2_sb = kqq_pool.tile([128, ROWS], bf16)
            # V (fp8) row-layout: per (seg, jt) a 272-col block
            # (256 V + ones col at 256 + pad).
            v_sb = v_pool.tile([128, SEGS * NJT * VST], f8)

            # load qT / hT as 8 tiles each of [128, 1024]
            qts = {}
            hts = {}
            for e in range(2):
                for c in range(ROWS // 1024):
                    t = qh_pool.tile([128, 1024], bf16, tag=f"qt{e}_{c}")
                    nc.sync.dma_start(
                        t[:], qT_a[e * 128:(e + 1) * 128,
                                   c * 1024:(c + 1) * 1024])
                    qts[(e, c)] = t
                    t2 = qh_pool.tile([128, 1024], bf16, tag=f"ht{e}_{c}")
                    nc.sync.dma_start(
                        t2[:], hT_a[e * 128:(e + 1) * 128,
                                    c * 1024:(c + 1) * 1024])
                    hts[(e, c)] = t2

            def _slice(tiles, e, col, width):
                c, off = col // 1024, col % 1024
                assert off + width <= 1024
                return tiles[(e, c)][:, off:off + width]

            # ---------------- pipelined main loop ----------------
            chunks = [(s, ic) for s in range(SEGS) for ic in range(NIC)]

            def emit_kt_half(pp_kt, c, half):
                # hT cols [c*1024+half*512, +512) = key tiles 8c+4h..+3
                ps = pp_kt.tile([64, 512], f32, tag="kt")
                col = c * 1024 + half * 512
                for e in range(2):
                    nc.tensor.matmul(
                        ps[:],
                        wk_sb[:, e * QD:(e + 1) * QD],
                        _slice(hts, e, col, 512),
                        start=(e == 0), stop=(e == 1))
                # evac: even tiles -> partitions 0-63, odd -> 64-127
                src = ps[:].rearrange("p (n c) -> p n c", c=256)
                base = c * 512 + half * 256
                for par in range(2):
                    dst = kT2_sb[64 * par:64 * (par + 1), base:base + 256]
                    nc.vector.tensor_copy(
                        dst.rearrange("p (n c) -> p n c", c=128),
                        src[:, :, 128 * par:128 * (par + 1)])

            def emit_qq_chunk(pp_qq, c512):
                # qT cols [c512*512, +512), duplicated via column tiling
                ps = pp_qq.tile([128, 512], f32, tag="qq")
                col = c512 * 512
                for e in range(2):
                    nc.tensor.matmul(
                        ps[0:64, :],
                        wq_sb[:, e * QD:(e + 1) * QD],
                        _slice(qts, e, col, 512),
                        start=(e == 0), stop=(e == 1))
                for e in range(2):
                    nc.tensor.matmul(
                        ps[64:128, :],
                        wq_sb[:, e * QD:(e + 1) * QD],
                        _slice(qts, e, col, 512),
                        start=(e == 0), stop=(e == 1),
                        tile_position=(0, 64))
                nc.vector.tensor_copy(qq2_sb[:, col:col + 512], ps[:])

            def emit_v_block(pp_v, s, jt):
                ps = pp_v.tile([128, HID], f32, tag="v")
                col = s * LH + jt * 128
                for e in range(2):
                    nc.tensor.matmul(
                        ps[:],
                        _slice(hts, e, col, 128),
                        wv_sb[:, e * HID:(e + 1) * HID],
                        start=(e == 0), stop=(e == 1))
                base = (s * NJT + jt) * VST
                nc.vector.tensor_copy(v_sb[:, base:base + HID], ps[:])
                nc.vector.memset(v_sb[:, base + HID:base + HID + 1], 1.0)

            def scores_beat(state, k):
                s, ic = state["c"]
                icol = s * LQ + ic * ICW
                st = ps_st.tile([128, 2048], f32, tag="st")
                kcol = s * (LH // 2) + k * 128
                for h in range(2):
                    nc.tensor.matmul(
                        st[:, h * 512:(h + 1) * 512],
                        kT2_sb[0:64, kcol:kcol + 128],
                        qq2_sb[0:64, icol + h * 512:icol + (h + 1) * 512],
                        start=True, stop=True)
                for h in range(2):
                    nc.tensor.matmul(
                        st[:, 1024 + h * 512:1024 + (h + 1) * 512],
                        kT2_sb[64:128, kcol:kcol + 128],
                        qq2_sb[64:128,
                               icol + h * 512:icol + (h + 1) * 512],
                        start=True, stop=True)
                pt2 = pt_pool.tile([128, 2048], f8, tag="pt")
                nc.scalar.activation(pt2[:], st[:], AF.Exp,
                                     scale=SCALE, bias=nb3_sb[:])
                state["pts"].append(pt2)
                # prefetch q rows for this chunk's epilogue
                row0 = icol + k * 128
                qt = q_pool.tile([128, HID], f32, tag="q")
                nc.sync.dma_start(qt[:], q_a[row0:row0 + 128, :])
                state["qts"].append(qt)

            def att_beat(ps_att, state, il):
                s, ic = state["c"]
                att = ps_att.tile([128, 512], f32, tag="att")
                for jp in range(NJP):
                    lhsT = (state["pts"][jp][:]
                            .rearrange("p (two q) -> p two q", two=2)
                            [:, :, il * 128:(il + 1) * 128])
                    vb = 2 * (s * NJP + jp)
                    rhs = (v_sb[:]
                           .rearrange("p (n c) -> p n c", c=VST)
                           [:, vb:vb + 2, 0:HID + 1])
                    nc.tensor.matmul(att[:, 0:HID + 1], lhsT, rhs,
                                     start=(jp == 0), stop=(jp == NJP - 1),
                                     perf_mode=DR)
                qt = state["qts"][il]
                rden = st8_pool.tile([128, 1], f32, tag="rd")
                nc.vector.reciprocal(rden[:], att[:, HID:HID + 1])
                x0 = ep8_pool.tile([128, HID], bf16, tag="x0")
                nc.vector.scalar_tensor_tensor(
                    x0[:], att[:, 0:HID], rden[:].opt(), qt[:],
                    op0=Alu.mult, op1=Alu.add)
                mv6 = st8_pool.tile([128, 6], f32, tag="mv6")
                nc.vector.bn_stats(mv6[:], x0[:])
                nc.vector.bn_aggr(state["mva0"][:, 2 * il:2 * il + 2],
                                  mv6[:])
                state["xs"].append(x0)

            def att_finish(ps_fc, ps_tp, state):
                s, ic = state["c"]
                mva0 = state["mva0"]
                ln8a = st8_pool.tile([128, NIL], f32, tag="ln8a")
                nc.scalar.activation(
                    ln8a[:].rearrange("p (t o) -> p t o", o=1),
                    mva0[:].rearrange("p (t o) -> p t o", o=2)[:, :, 1:2],
                    AF.Ln, bias=eps_sb[:])
                rstd8a = st8_pool.tile([128, NIL], f32, tag="r8a")
                nc.scalar.activation(rstd8a[:], ln8a[:], AF.Exp,
                                     scale=-0.5)

                mva1 = st8_pool.tile([128, 2 * NIL], f32, tag="mva1")
                ys = []
                for il in range(NIL):
                    x0 = state["xs"][il]
                    z = ep_pool.tile([128, HID], bf16, tag="z")
                    nc.gpsimd.tensor_scalar(
                        z[:], x0[:], mva0[:, 2 * il:2 * il + 1].opt(),
                        rstd8a[:, il:il + 1].opt(),
                        op0=Alu.subtract, op1=Alu.mult)
                    if apply0:
                        z2 = ep_pool.tile([128, HID], bf16, tag="z2")
                        nc.gpsimd.tensor_tensor(z2[:], z[:], n0w_sb[:],
                                                op=Alu.mult)
                        z3 = ep_pool.tile([128, HID], bf16, tag="z3")
                        nc.gpsimd.tensor_tensor(z3[:], z2[:], n0b_sb[:],
                                                op=Alu.add)
                        zf = z3
                    else:
                        zf = z
                    tp = ps_tp.tile([128, 256], bf16, tag="tp")
                    for hh in range(2):
                        nc.tensor.transpose(
                            tp[:, hh * 128:(hh + 1) * 128],
                            zf[:, hh * 128:(hh + 1) * 128],
                            idt_sb[:])
                    zT = ep_pool.tile([128, 256], bf16, tag="zT")
                    nc.vector.tensor_copy(zT[:], tp[:])
                    hres = ps_fc.tile([128, HID], f32, tag="fc")
                    nc.tensor.matmul(hres[:], one_sb[:], fb_sb[:],
                                     start=True, stop=False)
                    for hh in range(2):
                        nc.tensor.matmul(
                            hres[:], zT[:, hh * 128:(hh + 1) * 128],
                            fw_sb[:, hh * HID:(hh + 1) * HID],
                            start=False, stop=(hh == 1))
                    y0 = ep8_pool.tile([128, HID], bf16, tag="y0")
                    nc.vector.scalar_tensor_tensor(
                        y0[:], hres[:], 0.0, zf[:],
                        op0=Alu.max, op1=Alu.add)
                    mv6b = st8_pool.tile([128, 6], f32, tag="mv6b")
                    nc.vector.bn_stats(mv6b[:], y0[:])
                    nc.vector.bn_aggr(mva1[:, 2 * il:2 * il + 2],
                                      mv6b[:])
                    ys.append(y0)

                ln8b = st8_pool.tile([128, NIL], f32, tag="ln8b")
                nc.scalar.activation(
                    ln8b[:].rearrange("p (t o) -> p t o", o=1),
                    mva1[:].rearrange("p (t o) -> p t o", o=2)[:, :, 1:2],
                    AF.Ln, bias=eps_sb[:])
                rstd8b = st8_pool.tile([128, NIL], f32, tag="r8b")
                nc.scalar.activation(rstd8b[:], ln8b[:], AF.Exp,
                                     scale=-0.5)

                icol = s * LQ + ic * ICW
                for il in range(NIL):
                    row0 = icol + il * 128
                    ot = o_pool.tile([128, HID], f32, tag="ot")
                    nc.gpsimd.tensor_scalar(
                        ot[:], ys[il][:], mva1[:, 2 * il:2 * il + 1].opt(),
                        rstd8b[:, il:il + 1].opt(),
                        op0=Alu.subtract, op1=Alu.mult)
                    nc.sync.dma_start(out_a[row0:row0 + 128, :], ot[:])

            def new_state(c):
                return {"c": c, "pts": [], "qts": [], "xs": [],
                        "mva0": st8_pool.tile([128, 2 * NIL], f32,
                                              tag="mva0", name="mva0")}

            # Stage 0: scores(c0) interleaved with the projections so the
            # PE is never idle while exp(c0) streams on the scalar engine.
            cur = new_state(chunks[0])
            with (
                tc.tile_pool(name="pp_kt", bufs=1,
                             space=bass.MemorySpace.PSUM) as pp_kt,
                tc.tile_pool(name="pp_qq", bufs=1,
                             space=bass.MemorySpace.PSUM) as pp_qq,
                tc.tile_pool(name="pp_v", bufs=2,
                             space=bass.MemorySpace.PSUM) as pp_v,
            ):
                for c, h in ((0, 0), (0, 1), (1, 0), (1, 1)):
                    emit_kt_half(pp_kt, c, h)
                emit_qq_chunk(pp_qq, 0)
                emit_qq_chunk(pp_qq, 1)
                filler = ([("kt", (2, 0)), ("kt", (2, 1)),
                           ("kt", (3, 0)), ("kt", (3, 1))]
                          + [("qq", i) for i in range(2, 8)]
                          + [("v", (s, jt)) for s in range(SEGS)
                             for jt in range(NJT)])
                fi = 0
                for k in range(NJP):
                    scores_beat(cur, k)
                    for _ in range(6):
                        if fi < len(filler):
                            kind, arg = filler[fi]
                            fi += 1
                            if kind == "kt":
                                emit_kt_half(pp_kt, *arg)
                            elif kind == "qq":
                                emit_qq_chunk(pp_qq, arg)
                            else:
                                emit_v_block(pp_v, *arg)
                while fi < len(filler):
                    kind, arg = filler[fi]
                    fi += 1
                    if kind == "kt":
                        emit_kt_half(pp_kt, *arg)
                    elif kind == "qq":
                        emit_qq_chunk(pp_qq, arg)
                    else:
                        emit_v_block(pp_v, *arg)
            prev = cur

            with (
                tc.tile_pool(name="ps_att", bufs=2,
                             space=bass.MemorySpace.PSUM) as ps_att,
                tc.tile_pool(name="ps_fc", bufs=1,
                             space=bass.MemorySpace.PSUM) as ps_fc,
                tc.tile_pool(name="ps_tp", bufs=1,
                             space=bass.MemorySpace.PSUM) as ps_tp,
            ):
                for c in chunks[1:]:
                    cur = new_state(c)
                    for k in range(NJP):
                        scores_beat(cur, k)
                        att_beat(ps_att, prev, k)
                    att_finish(ps_fc, ps_tp, prev)
                    prev = cur
                for k in range(NJP):
                    att_beat(ps_att, prev, k)
                att_finish(ps_fc, ps_tp, prev)

    nc.compile()
    return nc


def _get_nc(apply0: bool):
    key = (bool(apply0),)
    if key not in _built:
        _built[key] = _build(apply0)
    return _built[key]


def _shard(inputs, apply0):
    from concourse import mybir
    bf = mybir.dt.np(mybir.dt.bfloat16)

    q = np.ascontiguousarray(np.asarray(inputs["q"], dtype=np.float32))
    h = np.ascontiguousarray(np.asarray(inputs["h"], dtype=np.float32))
    WQ = np.asarray(inputs["WQ"], dtype=np.float32)
    WK = np.asarray(inputs["WK"], dtype=np.float32)
    WV = np.asarray(inputs["WV"], dtype=np.float32)
    fcw = np.asarray(inputs["fc_w"], dtype=np.float32)
    fcb = np.asarray(inputs["fc_b"], dtype=np.float32)

    WQT = np.ascontiguousarray(WQ.T).astype(bf)
    WKT = np.ascontiguousarray(WK.T).astype(bf)
    WVT = np.ascontiguousarray(WV.T).astype(bf)
    FCWT = np.ascontiguousarray(fcw.T).astype(bf)
    FCB = np.ascontiguousarray(fcb.reshape(1, HID)).astype(bf)
    IDT = np.eye(128, dtype=np.float32).astype(bf)

    in_maps = []
    for c in range(NCORES):
        sl = slice(c * ROWS, (c + 1) * ROWS)
        m = {
            "qT": np.ascontiguousarray(q[sl].T).astype(bf),
            "q": q[sl],
            "hT": np.ascontiguousarray(h[sl].T).astype(bf),
            "WQT": WQT, "WKT": WKT, "WVT": WVT,
            "FCWT": FCWT, "FCB": FCB, "IDT": IDT,
        }
        if apply0:
            m["N0W"] = np.ascontiguousarray(
                np.broadcast_to(np.asarray(inputs["norm0_w"], np.float32),
                                (128, HID)))
            m["N0B"] = np.ascontiguousarray(
                np.broadcast_to(np.asarray(inputs["norm0_b"], np.float32),
                                (128, HID)))
        in_maps.append(m)
    return in_maps


def _run(inputs, trace=False, tmpdir=None):
    from concourse import bass_utils

    n0w = np.asarray(inputs["norm0_w"], np.float32)
    n0b = np.asarray(inputs["norm0_b"], np.float32)
    n1w = np.asarray(inputs["norm1_w"], np.float32)
    n1b = np.asarray(inputs["norm1_b"], np.float32)
    apply0 = not (np.allclose(n0w, 1.0) and np.allclose(n0b, 0.0))
    apply1 = not (np.allclose(n1w, 1.0) and np.allclose(n1b, 0.0))

    nc = _get_nc(apply0)
    in_maps = _shard(inputs, apply0)
    res = bass_utils.run_bass_kernel_spmd(
        nc, in_maps, core_ids=list(range(NCORES)), trace=trace,
        tmpdir=tmpdir)
    out = np.concatenate([np.asarray(res.results[c]["out"])
                          for c in range(NCORES)], axis=0)
    if apply1:
        out = out * n1w[None, :] + n1b[None, :]
    return out.astype(np.float32), res


def kernel(**inputs):
    out, _ = _run(inputs, trace=False)
    return out


# revision 9
# speedup vs baseline: 1.4660x; 1.4660x over previous
"""Trainium2 Bass kernel for nn_AttentionBlock (ragged_sequence, 16 equal
segments of 2048 q/kv tokens, HID=256, QD=64) on 8 NeuronCores.

Sharding: 2 segments (4096 rows) per core, weights replicated, outputs
concatenated host-side (attention is block-diagonal per segment -> no
cross-core communication needed).

v2: software-pipelined scores/exp vs att/epilogue, fp8 P+V with DoubleRow
att matmuls, wide exp slices, gpsimd epilogue offload.
"""

import os
import sys

os.environ.setdefault("MYCRO_LOCAL_CACHE", "1")
if "/opt/trn_rl_repo" not in sys.path:
    sys.path.insert(0, "/opt/trn_rl_repo")

import numpy as np

HID = 256
QD = 64
LQ = 2048
LH = 2048
B = 16
NCORES = 8
SEGS = 2                  # segments per core
ROWS = SEGS * LQ          # 4096 q rows per core
EPS = 1e-5
SCALE = 1.0 / 8.0         # 1/sqrt(QD)
NJT = LH // 128           # 16 key tiles per segment
NJP = NJT // 2            # 8 key-tile pairs per segment
NIC = 2                   # 1024-col query chunks per segment
ICW = LQ // NIC           # 1024
NIL = ICW // 128          # 8 query row-tiles per chunk
VST = 272                 # fp8 V block stride (256 V + 1 ones + pad, 16-aligned)

_built = {}


def _patch_act_tables():
    """Make the act-table pass choose the combined exp+ln table for every
    activation: blank all other tables (indices preserved so walrus's
    act_func_set_id remap stays correct). Avoids 100+ ACT_TABLE_LOADs
    (1.28us each) from alternating Exp/Ln table picks."""
    import functools
    import concourse.hw_specs as hw_specs
    import concourse.bacc as bacc_mod
    if getattr(hw_specs, "_attn_tables_patched", False):
        return
    orig = hw_specs.get_activation_tables

    @functools.cache
    def patched(arch):
        tabs = dict(orig(arch))
        joint = "natural_log_exp_and_others"
        assert joint in tabs, sorted(tabs)
        return {name: (funcs if name == joint else set())
                for name, funcs in tabs.items()}

    hw_specs.get_activation_tables = patched
    bacc_mod.get_activation_tables = patched
    hw_specs._attn_tables_patched = True


def _build(apply0: bool):
    from concourse import bacc, bass, mybir, tile

    _patch_act_tables()

    dt = mybir.dt
    f32 = dt.float32
    bf16 = dt.bfloat16
    f8 = dt.float8e4
    AF = mybir.ActivationFunctionType
    Alu = mybir.AluOpType
    DR = mybir.MatmulPerfMode.DoubleRow

    nc = bacc.Bacc("TRN2", target_bir_lowering=False, debug=False,
                   enable_asserts=False)

    qT_d = nc.dram_tensor("qT", [HID, ROWS], bf16, kind="ExternalInput")
    q_d = nc.dram_tensor("q", [ROWS, HID], f32, kind="ExternalInput")
    hT_d = nc.dram_tensor("hT", [HID, ROWS], bf16, kind="ExternalInput")
    wqT_d = nc.dram_tensor("WQT", [HID, QD], bf16, kind="ExternalInput")
    wkT_d = nc.dram_tensor("WKT", [HID, QD], bf16, kind="ExternalInput")
    wvT_d = nc.dram_tensor("WVT", [HID, HID], bf16, kind="ExternalInput")
    fwT_d = nc.dram_tensor("FCWT", [HID, HID], bf16, kind="ExternalInput")
    fb_d = nc.dram_tensor("FCB", [1, HID], bf16, kind="ExternalInput")
    idt_d = nc.dram_tensor("IDT", [128, 128], bf16, kind="ExternalInput")
    if apply0:
        n0w_d = nc.dram_tensor("N0W", [128, HID], f32, kind="ExternalInput")
        n0b_d = nc.dram_tensor("N0B", [128, HID], f32, kind="ExternalInput")
    out_d = nc.dram_tensor("out", [ROWS, HID], f32, kind="ExternalOutput")

    qT_a, q_a, hT_a = qT_d.ap(), q_d.ap(), hT_d.ap()
    out_a = out_d.ap()

    with tile.TileContext(nc) as tc:
        with (
            tc.tile_pool(name="const", bufs=1) as cpool,
            tc.tile_pool(name="kqq", bufs=1) as kqq_pool,
            tc.tile_pool(name="vsb", bufs=1) as v_pool,
            tc.tile_pool(name="qhT", bufs=1) as qh_pool,
            tc.tile_pool(name="pt", bufs=18) as pt_pool,
            tc.tile_pool(name="qrow", bufs=18) as q_pool,
            tc.tile_pool(name="ep", bufs=4) as ep_pool,
            tc.tile_pool(name="ep8", bufs=18) as ep8_pool,
            tc.tile_pool(name="st8", bufs=8) as st8_pool,
            tc.tile_pool(name="outp", bufs=6) as o_pool,
            tc.tile_pool(name="ps_st", bufs=1,
                         space=bass.MemorySpace.PSUM) as ps_st,
        ):
            # ---- constants ----
            wq_sb = cpool.tile([128, 2 * QD], bf16)     # [e, (chunk, c)]
            wk_sb = cpool.tile([128, 2 * QD], bf16)
            wv_sb = cpool.tile([128, 2 * HID], bf16)    # [e, (chunk, d)]
            fw_sb = cpool.tile([128, 2 * HID], bf16)    # fc_w.T chunks
            fb_sb = cpool.tile([1, HID], bf16)
            one_sb = cpool.tile([1, 128], bf16)
            idt_sb = cpool.tile([128, 128], bf16)
            for e in range(2):
                nc.sync.dma_start(wq_sb[:, e * QD:(e + 1) * QD],
                                  wqT_d.ap()[e * 128:(e + 1) * 128, :])
                nc.sync.dma_start(wk_sb[:, e * QD:(e + 1) * QD],
                                  wkT_d.ap()[e * 128:(e + 1) * 128, :])
                nc.sync.dma_start(wv_sb[:, e * HID:(e + 1) * HID],
                                  wvT_d.ap()[e * 128:(e + 1) * 128, :])
                nc.sync.dma_start(fw_sb[:, e * HID:(e + 1) * HID],
                                  fwT_d.ap()[e * 128:(e + 1) * 128, :])
            nc.sync.dma_start(fb_sb[:], fb_d.ap()[:, :])
            nc.sync.dma_start(idt_sb[:], idt_d.ap()[:, :])
            nc.vector.memset(one_sb[:], 1.0)
            eps_sb = cpool.tile([128, 1], f32)
            nc.vector.memset(eps_sb[:], EPS)
            nb3_sb = cpool.tile([128, 1], f32)
            nc.vector.memset(nb3_sb[:], -3.0)
            if apply0:
                n0w_sb = cpool.tile([128, HID], f32)
                n0b_sb = cpool.tile([128, HID], f32)
                nc.sync.dma_start(n0w_sb[:], n0w_d.ap()[:, :])
                nc.sync.dma_start(n0b_sb[:], n0b_d.ap()[:, :])

            # persistent activations
            # kT2: row-tiled K^T. partitions 0-63: even key tiles, 64-127:
            # odd key tiles; per seg 8 pair-blocks of 128 cols.
            kT2_sb = kqq_pool.tile([128, SEGS * LH // 2], bf16)
            # qq2: qq^T duplicated into both partition halves.
            qq2_sb = kqq_pool.tile([128, ROWS], bf16)
            # V (fp8) row-layout: per (seg, jt) a 272-col block
            # (256 V + ones col at 256 + pad).
            v_sb = v_pool.tile([128, SEGS * NJT * VST], f8)

            # load qT / hT as 8 tiles each of [128, 1024]
            qts = {}
            hts = {}
            for e in range(2):
                for c in range(ROWS // 1024):
                    t = qh_pool.tile([128, 1024], bf16, tag=f"qt{e}_{c}")
                    nc.sync.dma_start(
                        t[:], qT_a[e * 128:(e + 1) * 128,
                                   c * 1024:(c + 1) * 1024])
                    qts[(e, c)] = t
                    t2 = qh_pool.tile([128, 1024], bf16, tag=f"ht{e}_{c}")
                    nc.sync.dma_start(
                        t2[:], hT_a[e * 128:(e + 1) * 128,
                                    c * 1024:(c + 1) * 1024])
                    hts[(e, c)] = t2

            def _slice(tiles, e, col, width):
                c, off = col // 1024, col % 1024
                assert off + width <= 1024
                return tiles[(e, c)][:, off:off + width]

            # ---------------- pipelined main loop ----------------
            chunks = [(s, ic) for s in range(SEGS) for ic in range(NIC)]

            def emit_kt_half(pp_kt, c, half):
                # hT cols [c*1024+half*512, +512) = key tiles 8c+4h..+3
                ps = pp_kt.tile([64, 512], f32, tag="kt")
                col = c * 1024 + half * 512
                for e in range(2):
                    nc.tensor.matmul(
                        ps[:],
                        wk_sb[:, e * QD:(e + 1) * QD],
                        _slice(hts, e, col, 512),
                        start=(e == 0), stop=(e == 1))
                # evac: even tiles -> partitions 0-63, odd -> 64-127
                src = ps[:].rearrange("p (n c) -> p n c", c=256)
                base = c * 512 + half * 256
                for par in range(2):
                    dst = kT2_sb[64 * par:64 * (par + 1), base:base + 256]
                    nc.vector.tensor_copy(
                        dst.rearrange("p (n c) -> p n c", c=128),
                        src[:, :, 128 * par:128 * (par + 1)])

            def emit_qq_chunk(pp_qq, c512):
                # qT cols [c512*512, +512), duplicated via column tiling
                ps = pp_qq.tile([128, 512], f32, tag="qq")
                col = c512 * 512
                for e in range(2):
                    nc.tensor.matmul(
                        ps[0:64, :],
                        wq_sb[:, e * QD:(e + 1) * QD],
                        _slice(qts, e, col, 512),
                        start=(e == 0), stop=(e == 1))
                for e in range(2):
                    nc.tensor.matmul(
                        ps[64:128, :],
                        wq_sb[:, e * QD:(e + 1) * QD],
                        _slice(qts, e, col, 512),
                        start=(e == 0), stop=(e == 1),
                        tile_position=(0, 64))
                nc.vector.tensor_copy(qq2_sb[:, col:col + 512], ps[:])

            def emit_v_block(pp_v, s, jt):
                ps = pp_v.tile([128, HID], f32, tag="v")
                col = s * LH + jt * 128
                for e in range(2):
                    nc.tensor.matmul(
                        ps[:],
                        _slice(hts, e, col, 128),
                        wv_sb[:, e * HID:(e + 1) * HID],
                        start=(e == 0), stop=(e == 1))
                base = (s * NJT + jt) * VST
                nc.vector.tensor_copy(v_sb[:, base:base + HID], ps[:])
                nc.vector.memset(v_sb[:, base + HID:base + HID + 1], 1.0)

            def scores_beat(state, k):
                s, ic = state["c"]
                icol = s * LQ + ic * ICW
                st = ps_st.tile([128, 2048], f32, tag="st")
                kcol = s * (LH // 2) + k * 128
                for h in range(2):
                    nc.tensor.matmul(
                        st[:, h * 512:(h + 1) * 512],
                        kT2_sb[0:64, kcol:kcol + 128],
                        qq2_sb[0:64, icol + h * 512:icol + (h + 1) * 512],
                        start=True, stop=True)
                    nc.tensor.matmul(
                        st[:, 1024 + h * 512:1024 + (h + 1) * 512],
                        kT2_sb[64:128, kcol:kcol + 128],
                        qq2_sb[64:128,
                               icol + h * 512:icol + (h + 1) * 512],
                        start=True, stop=True)
                pt2 = pt_pool.tile([128, 2048], f8, tag="pt")
                nc.scalar.activation(pt2[:], st[:], AF.Exp,
                                     scale=SCALE, bias=nb3_sb[:])
                state["pts"].append(pt2)
                # prefetch q rows for this chunk's epilogue
                row0 = icol + k * 128
                qt = q_pool.tile([128, HID], f32, tag="q")
                nc.sync.dma_start(qt[:], q_a[row0:row0 + 128, :])
                state["qts"].append(qt)

            def att_beat(ps_att, state, il):
                s, ic = state["c"]
                att = ps_att.tile([128, 512], f32, tag="att")
                for jp in range(NJP):
                    lhsT = (state["pts"][jp][:]
                            .rearrange("p (two q) -> p two q", two=2)
                            [:, :, il * 128:(il + 1) * 128])
                    vb = 2 * (s * NJP + jp)
                    rhs = (v_sb[:]
                           .rearrange("p (n c) -> p n c", c=VST)
                           [:, vb:vb + 2, 0:HID + 1])
                    nc.tensor.matmul(att[:, 0:HID + 1], lhsT, rhs,
                                     start=(jp == 0), stop=(jp == NJP - 1),
                                     perf_mode=DR)
                qt = state["qts"][il]
                rden = st8_pool.tile([128, 1], f32, tag="rd")
                nc.vector.reciprocal(rden[:], att[:, HID:HID + 1])
                x0 = ep8_pool.tile([128, HID], bf16, tag="x0")
                nc.vector.scalar_tensor_tensor(
                    x0[:], att[:, 0:HID], rden[:].opt(), qt[:],
                    op0=Alu.mult, op1=Alu.add)
                mv6 = st8_pool.tile([128, 6], f32, tag="mv6")
                nc.vector.bn_stats(mv6[:], x0[:])
                nc.vector.bn_aggr(state["mva0"][:, 2 * il:2 * il + 2],
                                  mv6[:])
                state["xs"].append(x0)

            def att_finish(ps_fc, ps_tp, state):
                s, ic = state["c"]
                mva0 = state["mva0"]
                ln8a = st8_pool.tile([128, NIL], f32, tag="ln8a")
                nc.scalar.activation(
                    ln8a[:].rearrange("p (t o) -> p t o", o=1),
                    mva0[:].rearrange("p (t o) -> p t o", o=2)[:, :, 1:2],
                    AF.Ln, bias=eps_sb[:])
                rstd8a = st8_pool.tile([128, NIL], f32, tag="r8a")
                nc.scalar.activation(rstd8a[:], ln8a[:], AF.Exp,
                                     scale=-0.5)

                mva1 = st8_pool.tile([128, 2 * NIL], f32, tag="mva1")
                ys = []
                for g in range(2):
                    zfs = []
                    for il4 in range(4):
                        il = g * 4 + il4
                        x0 = state["xs"][il]
                        z = ep_pool.tile([128, HID], bf16, tag="z")
                        nc.vector.tensor_scalar(
                            z[:], x0[:], mva0[:, 2 * il:2 * il + 1].opt(),
                            rstd8a[:, il:il + 1].opt(),
                            op0=Alu.subtract, op1=Alu.mult)
                        if apply0:
                            z2 = ep_pool.tile([128, HID], bf16, tag="z2")
                            nc.gpsimd.tensor_tensor(z2[:], z[:], n0w_sb[:],
                                                    op=Alu.mult)
                            z3 = ep_pool.tile([128, HID], bf16, tag="z3")
                            nc.gpsimd.tensor_tensor(z3[:], z2[:], n0b_sb[:],
                                                    op=Alu.add)
                            zfs.append(z3)
                        else:
                            zfs.append(z)
                    # batched transposes (one PE mode switch per group)
                    tp = ps_tp.tile([128, 1024], bf16, tag="tp")
                    for il4 in range(4):
                        for hh in range(2):
                            nc.tensor.transpose(
                                tp[:, il4 * 256 + hh * 128:
                                   il4 * 256 + (hh + 1) * 128],
                                zfs[il4][:, hh * 128:(hh + 1) * 128],
                                idt_sb[:])
                    zT = ep_pool.tile([128, 1024], bf16, tag="zT")
                    nc.vector.tensor_copy(zT[:], tp[:])
                    for il4 in range(4):
                        il = g * 4 + il4
                        hres = ps_fc.tile([128, HID], f32, tag="fc")
                        nc.tensor.matmul(hres[:], one_sb[:], fb_sb[:],
                                         start=True, stop=False)
                        for hh in range(2):
                            nc.tensor.matmul(
                                hres[:],
                                zT[:, il4 * 256 + hh * 128:
                                   il4 * 256 + (hh + 1) * 128],
                                fw_sb[:, hh * HID:(hh + 1) * HID],
                                start=False, stop=(hh == 1))
                        y0 = ep8_pool.tile([128, HID], bf16, tag="y0")
                        nc.vector.scalar_tensor_tensor(
                            y0[:], hres[:], 0.0, zfs[il4][:],
                            op0=Alu.max, op1=Alu.add)
                        mv6b = st8_pool.tile([128, 6], f32, tag="mv6b")
                        nc.vector.bn_stats(mv6b[:], y0[:])
                        nc.vector.bn_aggr(mva1[:, 2 * il:2 * il + 2],
                                          mv6b[:])
                        ys.append(y0)

                ln8b = st8_pool.tile([128, NIL], f32, tag="ln8b")
                nc.scalar.activation(
                    ln8b[:].rearrange("p (t o) -> p t o", o=1),
                    mva1[:].rearrange("p (t o) -> p t o", o=2)[:, :, 1:2],
                    AF.Ln, bias=eps_sb[:])
                rstd8b = st8_pool.tile([128, NIL], f32, tag="r8b")
                nc.scalar.activation(rstd8b[:], ln8b[:], AF.Exp,
                                     scale=-0.5)

                icol = s * LQ + ic * ICW
                for il in range(NIL):
                    row0 = icol + il * 128
                    b1 = st8_pool.tile([128, 1], f32, tag="b1")
                    nc.vector.tensor_scalar(
                        b1[:], mva1[:, 2 * il:2 * il + 1],
                        rstd8b[:, il:il + 1].opt(), -1.0,
                        op0=Alu.mult, op1=Alu.mult)
                    ot = o_pool.tile([128, HID], f32, tag="ot")
                    nc.scalar.activation(
                        ot[:], ys[il][:], AF.Identity,
                        bias=b1[:], scale=rstd8b[:, il:il + 1].opt())
                    nc.sync.dma_start(out_a[row0:row0 + 128, :], ot[:])

            def new_state(c):
                return {"c": c, "pts": [], "qts": [], "xs": [],
                        "mva0": st8_pool.tile([128, 2 * NIL], f32,
                                              tag="mva0", name="mva0")}

            # Stage 0: scores(c0) interleaved with the projections so the
            # PE is never idle while exp(c0) streams on the scalar engine.
            cur = new_state(chunks[0])
            with (
                tc.tile_pool(name="pp_kt", bufs=1,
                             space=bass.MemorySpace.PSUM) as pp_kt,
                tc.tile_pool(name="pp_qq", bufs=1,
                             space=bass.MemorySpace.PSUM) as pp_qq,
                tc.tile_pool(name="pp_v", bufs=2,
                             space=bass.MemorySpace.PSUM) as pp_v,
            ):
                for c, h in ((0, 0), (0, 1), (1, 0), (1, 1)):
                    emit_kt_half(pp_kt, c, h)
                emit_qq_chunk(pp_qq, 0)
                emit_qq_chunk(pp_qq, 1)
                filler = ([("kt", (2, 0)), ("kt", (2, 1)),
                           ("kt", (3, 0)), ("kt", (3, 1))]
                          + [("qq", i) for i in range(2, 8)]
                          + [("v", (s, jt)) for s in range(SEGS)
                             for jt in range(NJT)])
                fi = 0
                for k in range(NJP):
                    scores_beat(cur, k)
                    for _ in range(6):
                        if fi < len(filler):
                            kind, arg = filler[fi]
                            fi += 1
                            if kind == "kt":
                                emit_kt_half(pp_kt, *arg)
                            elif kind == "qq":
                                emit_qq_chunk(pp_qq, arg)
                            else:
                                emit_v_block(pp_v, *arg)
                while fi < len(filler):
                    kind, arg = filler[fi]
                    fi += 1
                    if kind == "kt":
                        emit_kt_half(pp_kt, *arg)
                    elif kind == "qq":
                        emit_qq_chunk(pp_qq, arg)
                    else:
                        emit_v_block(pp_v, *arg)
            prev = cur

            with (
                tc.tile_pool(name="ps_att", bufs=2,
                             space=bass.MemorySpace.PSUM) as ps_att,
                tc.tile_pool(name="ps_fc", bufs=1,
                             space=bass.MemorySpace.PSUM) as ps_fc,
                tc.tile_pool(name="ps_tp", bufs=1,
                             space=bass.MemorySpace.PSUM) as ps_tp,
            ):
                for c in chunks[1:]:
                    cur = new_state(c)
                    for k in range(NJP):
                        scores_beat(cur, k)
                        att_beat(ps_att, prev, k)
                    att_finish(ps_fc, ps_tp, prev)
                    prev = cur
                for k in range(NJP):
                    att_beat(ps_att, prev, k)
                att_finish(ps_fc, ps_tp, prev)

    nc.compile()
    return nc


def _get_nc(apply0: bool):
    key = (bool(apply0),)
    if key not in _built:
        _built[key] = _build(apply0)
    return _built[key]


def _shard(inputs, apply0):
    from concourse import mybir
    bf = mybir.dt.np(mybir.dt.bfloat16)

    q = np.ascontiguousarray(np.asarray(inputs["q"], dtype=np.float32))
    h = np.ascontiguousarray(np.asarray(inputs["h"], dtype=np.float32))
    WQ = np.asarray(inputs["WQ"], dtype=np.float32)
    WK = np.asarray(inputs["WK"], dtype=np.float32)
    WV = np.asarray(inputs["WV"], dtype=np.float32)
    fcw = np.asarray(inputs["fc_w"], dtype=np.float32)
    fcb = np.asarray(inputs["fc_b"], dtype=np.float32)

    WQT = np.ascontiguousarray(WQ.T).astype(bf)
    WKT = np.ascontiguousarray(WK.T).astype(bf)
    WVT = np.ascontiguousarray(WV.T).astype(bf)
    FCWT = np.ascontiguousarray(fcw.T).astype(bf)
    FCB = np.ascontiguousarray(fcb.reshape(1, HID)).astype(bf)
    IDT = np.eye(128, dtype=np.float32).astype(bf)

    in_maps = []
    for c in range(NCORES):
        sl = slice(c * ROWS, (c + 1) * ROWS)
        m = {
            "qT": np.ascontiguousarray(q[sl].T).astype(bf),
            "q": q[sl],
            "hT": np.ascontiguousarray(h[sl].T).astype(bf),
            "WQT": WQT, "WKT": WKT, "WVT": WVT,
            "FCWT": FCWT, "FCB": FCB, "IDT": IDT,
        }
        if apply0:
            m["N0W"] = np.ascontiguousarray(
                np.broadcast_to(np.asarray(inputs["norm0_w"], np.float32),
                                (128, HID)))
            m["N0B"] = np.ascontiguousarray(
                np.broadcast_to(np.asarray(inputs["norm0_b"], np.float32),
                                (128, HID)))
        in_maps.append(m)
    return in_maps


def _run(inputs, trace=False, tmpdir=None):
    from concourse import bass_utils

    n0w = np.asarray(inputs["norm0_w"], np.float32)
    n0b = np.asarray(inputs["norm0_b"], np.float32)
    n1w = np.asarray(inputs["norm1_w"], np.float32)
    n1b = np.asarray(inputs["norm1_b"], np.float32)
    apply0 = not (np.allclose(n0w, 1.0) and np.allclose(n0b, 0.0))
    apply1 = not (np.allclose(n1w, 1.0) and np.allclose(n1b, 0.0))

    nc = _get_nc(apply0)
    in_maps = _shard(inputs, apply0)
    res = bass_utils.run_bass_kernel_spmd(
        nc, in_maps, core_ids=list(range(NCORES)), trace=trace,
        tmpdir=tmpdir)
    out = np.concatenate([np.asarray(res.results[c]["out"])
                          for c in range(NCORES)], axis=0)
    if apply1:
        out = out * n1w[None, :] + n1b[None, :]
    return out.astype(np.float32), res


def kernel(**inputs):
    out, _ = _run(inputs, trace=False)
    return out


# revision 14
# speedup vs baseline: 1.4714x; 1.0037x over previous
"""Trainium2 Bass kernel for nn_AttentionBlock (ragged_sequence, 16 equal
segments of 2048 q/kv tokens, HID=256, QD=64) on 8 NeuronCores.

Sharding: 2 segments (4096 rows) per core, weights replicated, outputs
concatenated host-side (attention is block-diagonal per segment -> no
cross-core communication needed).

v2: software-pipelined scores/exp vs att/epilogue, fp8 P+V with DoubleRow
att matmuls, wide exp slices, gpsimd epilogue offload.
"""

import os
import sys

os.environ.setdefault("MYCRO_LOCAL_CACHE", "1")
if "/opt/trn_rl_repo" not in sys.path:
    sys.path.insert(0, "/opt/trn_rl_repo")

import numpy as np

HID = 256
QD = 64
LQ = 2048
LH = 2048
B = 16
NCORES = 8
SEGS = 2                  # segments per core
ROWS = SEGS * LQ          # 4096 q rows per core
EPS = 1e-5
SCALE = 1.0 / 8.0         # 1/sqrt(QD)
NJT = LH // 128           # 16 key tiles per segment
NJP = NJT // 2            # 8 key-tile pairs per segment
NIC = 2                   # 1024-col query chunks per segment
ICW = LQ // NIC           # 1024
NIL = ICW // 128          # 8 query row-tiles per chunk
VST = 272                 # fp8 V block stride (256 V + 1 ones + pad, 16-aligned)

_built = {}


def _patch_act_tables():
    """Make the act-table pass choose the combined exp+ln table for every
    activation: blank all other tables (indices preserved so walrus's
    act_func_set_id remap stays correct). Avoids 100+ ACT_TABLE_LOADs
    (1.28us each) from alternating Exp/Ln table picks."""
    import functools
    import concourse.hw_specs as hw_specs
    import concourse.bacc as bacc_mod
    if getattr(hw_specs, "_attn_tables_patched", False):
        return
    orig = hw_specs.get_activation_tables

    @functools.cache
    def patched(arch):
        tabs = dict(orig(arch))
        joint = "natural_log_exp_and_others"
        assert joint in tabs, sorted(tabs)
        return {name: (funcs if name == joint else set())
                for name, funcs in tabs.items()}

    hw_specs.get_activation_tables = patched
    bacc_mod.get_activation_tables = patched
    hw_specs._attn_tables_patched = True


def _build(apply0: bool):
    from concourse import bacc, bass, mybir, tile

    _patch_act_tables()

    dt = mybir.dt
    f32 = dt.float32
    bf16 = dt.bfloat16
    f8 = dt.float8e4
    AF = mybir.ActivationFunctionType
    Alu = mybir.AluOpType
    DR = mybir.MatmulPerfMode.DoubleRow

    nc = bacc.Bacc("TRN2", target_bir_lowering=False, debug=False,
                   enable_asserts=False)

    qT_d = nc.dram_tensor("qT", [HID, ROWS], bf16, kind="ExternalInput")
    q_d = nc.dram_tensor("q", [ROWS, HID], f32, kind="ExternalInput")
    hT_d = nc.dram_tensor("hT", [HID, ROWS], bf16, kind="ExternalInput")
    wqT_d = nc.dram_tensor("WQT", [HID, QD], bf16, kind="ExternalInput")
    wkT_d = nc.dram_tensor("WKT", [HID, QD], bf16, kind="ExternalInput")
    wv8_d = nc.dram_tensor("WV8", [128, 2 * HID], f8, kind="ExternalInput")
    hT8_d = nc.dram_tensor("HT8", [128, 2 * ROWS], f8, kind="ExternalInput")
    fwT_d = nc.dram_tensor("FCWT", [HID, HID], bf16, kind="ExternalInput")
    fb_d = nc.dram_tensor("FCB", [1, HID], bf16, kind="ExternalInput")
    idt_d = nc.dram_tensor("IDT", [128, 128], bf16, kind="ExternalInput")
    if apply0:
        n0w_d = nc.dram_tensor("N0W", [128, HID], f32, kind="ExternalInput")
        n0b_d = nc.dram_tensor("N0B", [128, HID], f32, kind="ExternalInput")
    out_d = nc.dram_tensor("out", [ROWS, HID], f32, kind="ExternalOutput")

    qT_a, q_a, hT_a = qT_d.ap(), q_d.ap(), hT_d.ap()
    out_a = out_d.ap()

    with tile.TileContext(nc) as tc:
        with (
            tc.tile_pool(name="const", bufs=1) as cpool,
            tc.tile_pool(name="kqq", bufs=1) as kqq_pool,
            tc.tile_pool(name="vsb", bufs=1) as v_pool,
            tc.tile_pool(name="qhT", bufs=1) as qh_pool,
            tc.tile_pool(name="pt", bufs=18) as pt_pool,
            tc.tile_pool(name="qrow", bufs=18) as q_pool,
            tc.tile_pool(name="ep", bufs=4) as ep_pool,
            tc.tile_pool(name="ep8", bufs=18) as ep8_pool,
            tc.tile_pool(name="st8", bufs=8) as st8_pool,
            tc.tile_pool(name="outp", bufs=6) as o_pool,
            tc.tile_pool(name="ps_st", bufs=1,
                         space=bass.MemorySpace.PSUM) as ps_st,
        ):
            # ---- constants ----
            wq_sb = cpool.tile([128, 2 * QD], bf16)     # [e, (chunk, c)]
            wk_sb = cpool.tile([128, 2 * QD], bf16)
            wv8_sb = cpool.tile([128, 2 * HID], f8)     # V weights, e-pairs
            fw_sb = cpool.tile([128, 2 * HID], bf16)    # fc_w.T chunks
            fb_sb = cpool.tile([1, HID], bf16)
            one_sb = cpool.tile([1, 128], bf16)
            idt_sb = cpool.tile([128, 128], bf16)
            for e in range(2):
                nc.sync.dma_start(wq_sb[:, e * QD:(e + 1) * QD],
                                  wqT_d.ap()[e * 128:(e + 1) * 128, :])
                nc.sync.dma_start(wk_sb[:, e * QD:(e + 1) * QD],
                                  wkT_d.ap()[e * 128:(e + 1) * 128, :])
                nc.sync.dma_start(fw_sb[:, e * HID:(e + 1) * HID],
                                  fwT_d.ap()[e * 128:(e + 1) * 128, :])
            nc.sync.dma_start(wv8_sb[:], wv8_d.ap()[:, :])
            nc.sync.dma_start(fb_sb[:], fb_d.ap()[:, :])
            nc.sync.dma_start(idt_sb[:], idt_d.ap()[:, :])
            nc.vector.memset(one_sb[:], 1.0)
            eps_sb = cpool.tile([128, 1], f32)
            nc.vector.memset(eps_sb[:], EPS)
            nb3_sb = cpool.tile([128, 1], f32)
            nc.vector.memset(nb3_sb[:], -3.0)
            if apply0:
                n0w_sb = cpool.tile([128, HID], f32)
                n0b_sb = cpool.tile([128, HID], f32)
                nc.sync.dma_start(n0w_sb[:], n0w_d.ap()[:, :])
                nc.sync.dma_start(n0b_sb[:], n0b_d.ap()[:, :])

            # persistent activations
            # kT2: row-tiled K^T. partitions 0-63: even key tiles, 64-127:
            # odd key tiles; per seg 8 pair-blocks of 128 cols.
            kT2_sb = kqq_pool.tile([128, SEGS * LH // 2], bf16)
            # qq2: qq^T duplicated into both partition halves.
            qq2_sb = kqq_pool.tile([128, ROWS], bf16)
            # V (fp8) row-layout: per (seg, jt) a 272-col block
            # (256 V + ones col at 256 + pad).
            v_sb = v_pool.tile([128, SEGS * NJT * VST], f8)

            # fp8 e-paired h^T for the V projection (DoubleRow)
            ht8_sb = v_pool.tile([128, 2 * ROWS], f8)
            for c in range(4):
                nc.sync.dma_start(ht8_sb[:, c * 2048:(c + 1) * 2048],
                                  hT8_d.ap()[:, c * 2048:(c + 1) * 2048])

            # load qT / hT as 8 tiles each of [128, 1024]; hT first (kT
            # projection gates the first scores)
            qts = {}
            hts = {}
            for c in range(ROWS // 1024):
                for e in range(2):
                    t2 = qh_pool.tile([128, 1024], bf16, tag=f"ht{e}_{c}")
                    nc.sync.dma_start(
                        t2[:], hT_a[e * 128:(e + 1) * 128,
                                    c * 1024:(c + 1) * 1024])
                    hts[(e, c)] = t2
            for c in range(ROWS // 1024):
                for e in range(2):
                    t = qh_pool.tile([128, 1024], bf16, tag=f"qt{e}_{c}")
                    nc.sync.dma_start(
                        t[:], qT_a[e * 128:(e + 1) * 128,
                                   c * 1024:(c + 1) * 1024])
                    qts[(e, c)] = t

            def _slice(tiles, e, col, width):
                c, off = col // 1024, col % 1024
                assert off + width <= 1024
                return tiles[(e, c)][:, off:off + width]

            # ---------------- pipelined main loop ----------------
            chunks = [(s, ic) for s in range(SEGS) for ic in range(NIC)]

            def emit_kt_half(pp_kt, c, half):
                # hT cols [c*1024+half*512, +512) = key tiles 8c+4h..+3
                ps = pp_kt.tile([64, 512], f32, tag="kt")
                col = c * 1024 + half * 512
                for e in range(2):
                    nc.tensor.matmul(
                        ps[:],
                        wk_sb[:, e * QD:(e + 1) * QD],
                        _slice(hts, e, col, 512),
                        start=(e == 0), stop=(e == 1))
                # evac: even tiles -> partitions 0-63, odd -> 64-127
                src = ps[:].rearrange("p (n c) -> p n c", c=256)
                base = c * 512 + half * 256
                for par in range(2):
                    dst = kT2_sb[64 * par:64 * (par + 1), base:base + 256]
                    nc.vector.tensor_copy(
                        dst.rearrange("p (n c) -> p n c", c=128),
                        src[:, :, 128 * par:128 * (par + 1)])

            def emit_qq_chunk(pp_qq, c512):
                # qT cols [c512*512, +512), duplicated via column tiling
                ps = pp_qq.tile([128, 512], f32, tag="qq")
                col = c512 * 512
                for e in range(2):
                    nc.tensor.matmul(
                        ps[0:64, :],
                        wq_sb[:, e * QD:(e + 1) * QD],
                        _slice(qts, e, col, 512),
                        start=(e == 0), stop=(e == 1))
                for e in range(2):
                    nc.tensor.matmul(
                        ps[64:128, :],
                        wq_sb[:, e * QD:(e + 1) * QD],
                        _slice(qts, e, col, 512),
                        start=(e == 0), stop=(e == 1),
                        tile_position=(0, 64))
                nc.vector.tensor_copy(qq2_sb[:, col:col + 512], ps[:])

            def emit_v_pair(pp_v, s, jp):
                # two key tiles' V rows via one DoubleRow matmul each
                ps = pp_v.tile([128, 2 * HID], f32, tag="v")
                for u in range(2):
                    col = s * LH + (2 * jp + u) * 128
                    lhsT = (ht8_sb[:]
                            .rearrange("p (two r) -> p two r", two=2)
                            [:, :, col:col + 128])
                    rhs = (wv8_sb[:]
                           .rearrange("p (two d) -> p two d", two=2))
                    nc.tensor.matmul(ps[:, u * HID:(u + 1) * HID],
                                     lhsT, rhs, start=True, stop=True,
                                     perf_mode=DR)
                vb = 2 * (s * NJP + jp)
                dst = (v_sb[:].rearrange("p (n c) -> p n c", c=VST)
                       [:, vb:vb + 2, 0:HID])
                nc.vector.tensor_copy(
                    dst, ps[:].rearrange("p (n c) -> p n c", c=HID))

            def scores_beat(state, k):
                s, ic = state["c"]
                icol = s * LQ + ic * ICW
                if k == 0:
                    state["st"] = ps_st.tile([128, 2048], f32, tag="st",
                                             name="st")
                st = state["st"]
                kcol = s * (LH // 2) + k * 128
                for h in range(2):
                    nc.tensor.matmul(
                        st[:, h * 512:(h + 1) * 512],
                        kT2_sb[0:64, kcol:kcol + 128],
                        qq2_sb[0:64, icol + h * 512:icol + (h + 1) * 512],
                        start=True, stop=True)
                    nc.tensor.matmul(
                        st[:, 1024 + h * 512:1024 + (h + 1) * 512],
                        kT2_sb[64:128, kcol:kcol + 128],
                        qq2_sb[64:128,
                               icol + h * 512:icol + (h + 1) * 512],
                        start=True, stop=True)
                pt2 = pt_pool.tile([128, 2048], f8, tag="pt")
                nc.scalar.activation(pt2[:, 0:1024], st[:, 0:1024], AF.Exp,
                                     scale=SCALE, bias=nb3_sb[:])
                nc.scalar.activation(pt2[:, 1024:2048], st[:, 1024:2048],
                                     AF.Exp, scale=SCALE, bias=nb3_sb[:])
                state["pts"].append(pt2)
                # prefetch q rows for this chunk's epilogue
                row0 = icol + k * 128
                qt = q_pool.tile([128, HID], f32, tag="q")
                nc.sync.dma_start(qt[:], q_a[row0:row0 + 128, :])
                state["qts"].append(qt)

            def att_beat(ps_att, state, il):
                s, ic = state["c"]
                att = ps_att.tile([128, 512], f32, tag="att")
                for jp in range(NJP):
                    lhsT = (state["pts"][jp][:]
                            .rearrange("p (two q) -> p two q", two=2)
                            [:, :, il * 128:(il + 1) * 128])
                    vb = 2 * (s * NJP + jp)
                    rhs = (v_sb[:]
                           .rearrange("p (n c) -> p n c", c=VST)
                           [:, vb:vb + 2, 0:HID + 1])
                    nc.tensor.matmul(att[:, 0:HID + 1], lhsT, rhs,
                                     start=(jp == 0), stop=(jp == NJP - 1),
                                     perf_mode=DR)
                qt = state["qts"][il]
                rden = st8_pool.tile([128, 1], f32, tag="rd")
                nc.vector.reciprocal(rden[:], att[:, HID:HID + 1])
                x0 = state["xb"][:, il * HID:(il + 1) * HID]
                nc.vector.scalar_tensor_tensor(
                    x0, att[:, 0:HID], rden[:].opt(), qt[:],
                    op0=Alu.mult, op1=Alu.add)

            def finish_group(ps_fc, ps_tp, state, g):
                s, ic = state["c"]
                # batched LN0 stats for this group of 4 row-tiles
                mva0 = state["mva0"]
                mv24 = st8_pool.tile([128, 4 * 6], f32, tag="mv24")
                for il4 in range(4):
                    il = g * 4 + il4
                    nc.vector.bn_stats(
                        mv24[:, 6 * il4:6 * il4 + 6],
                        state["xb"][:, il * HID:(il + 1) * HID])
                    nc.vector.bn_aggr(mva0[:, 2 * il:2 * il + 2],
                                      mv24[:, 6 * il4:6 * il4 + 6])
                ln4a = st8_pool.tile([128, 4], f32, tag="ln4a")
                nc.scalar.activation(
                    ln4a[:].rearrange("p (t o) -> p t o", o=1),
                    mva0[:, 2 * g * 4:2 * (g + 1) * 4]
                    .rearrange("p (t o) -> p t o", o=2)[:, :, 1:2],
                    AF.Ln, bias=eps_sb[:])
                rstd4a = st8_pool.tile([128, 4], f32, tag="r4a")
                nc.scalar.activation(rstd4a[:], ln4a[:], AF.Exp,
                                     scale=-0.5)

                zfs = []
                for il4 in range(4):
                    il = g * 4 + il4
                    x0 = state["xb"][:, il * HID:(il + 1) * HID]
                    z = ep_pool.tile([128, HID], bf16, tag="z")
                    nc.vector.tensor_scalar(
                        z[:], x0, mva0[:, 2 * il:2 * il + 1].opt(),
                        rstd4a[:, il4:il4 + 1].opt(),
                        op0=Alu.subtract, op1=Alu.mult)
                    if apply0:
                        z2 = ep_pool.tile([128, HID], bf16, tag="z2")
                        nc.gpsimd.tensor_tensor(z2[:], z[:], n0w_sb[:],
                                                op=Alu.mult)
                        z3 = ep_pool.tile([128, HID], bf16, tag="z3")
                        nc.gpsimd.tensor_tensor(z3[:], z2[:], n0b_sb[:],
                                                op=Alu.add)
                        zfs.append(z3)
                    else:
                        zfs.append(z)
                # batched transposes (one PE mode switch per group)
                tp = ps_tp.tile([128, 1024], bf16, tag="tp")
                for il4 in range(4):
                    for hh in range(2):
                        nc.tensor.transpose(
                            tp[:, il4 * 256 + hh * 128:
                               il4 * 256 + (hh + 1) * 128],
                            zfs[il4][:, hh * 128:(hh + 1) * 128],
                            idt_sb[:])
                zT = ep_pool.tile([128, 1024], bf16, tag="zT")
                nc.vector.tensor_copy(zT[:], tp[:])
                for il4 in range(4):
                    il = g * 4 + il4
                    hres = ps_fc.tile([128, HID], f32, tag="fc")
                    nc.tensor.matmul(hres[:], one_sb[:], fb_sb[:],
                                     start=True, stop=False)
                    for hh in range(2):
                        nc.tensor.matmul(
                            hres[:],
                            zT[:, il4 * 256 + hh * 128:
                               il4 * 256 + (hh + 1) * 128],
                            fw_sb[:, hh * HID:(hh + 1) * HID],
                            start=False, stop=(hh == 1))
                    y0 = state["yb"][:, il * HID:(il + 1) * HID]
                    nc.vector.scalar_tensor_tensor(
                        y0, hres[:], 0.0, zfs[il4][:],
                        op0=Alu.max, op1=Alu.add)

                mva1 = state["mva1"]
                mv24b = st8_pool.tile([128, 4 * 6], f32, tag="mv24b")
                for il4 in range(4):
                    il = g * 4 + il4
                    nc.vector.bn_stats(
                        mv24b[:, 6 * il4:6 * il4 + 6],
                        state["yb"][:, il * HID:(il + 1) * HID])
                    nc.vector.bn_aggr(mva1[:, 2 * il:2 * il + 2],
                                      mv24b[:, 6 * il4:6 * il4 + 6])
                ln4b = st8_pool.tile([128, 4], f32, tag="ln4b")
                nc.scalar.activation(
                    ln4b[:].rearrange("p (t o) -> p t o", o=1),
                    mva1[:, 2 * g * 4:2 * (g + 1) * 4]
                    .rearrange("p (t o) -> p t o", o=2)[:, :, 1:2],
                    AF.Ln, bias=eps_sb[:])
                rstd4b = st8_pool.tile([128, 4], f32, tag="r4b")
                nc.scalar.activation(rstd4b[:], ln4b[:], AF.Exp,
                                     scale=-0.5)

                icol = s * LQ + ic * ICW
                for il4 in range(4):
                    il = g * 4 + il4
                    row0 = icol + il * 128
                    ot = o_pool.tile([128, HID], f32, tag="ot")
                    nc.vector.tensor_scalar(
                        ot[:], state["yb"][:, il * HID:(il + 1) * HID],
                        mva1[:, 2 * il:2 * il + 1].opt(),
                        rstd4b[:, il4:il4 + 1].opt(),
                        op0=Alu.subtract, op1=Alu.mult)
                    nc.sync.dma_start(out_a[row0:row0 + 128, :], ot[:])

            def new_state(c):
                return {"c": c, "pts": [], "qts": [],
                        "xb": ep8_pool.tile([128, NIL * HID], bf16,
                                            tag="xb", name="xb", bufs=2),
                        "yb": ep8_pool.tile([128, NIL * HID], bf16,
                                            tag="yb", name="yb", bufs=2),
                        "mva0": st8_pool.tile([128, 2 * NIL], f32,
                                              tag="mva0", name="mva0"),
                        "mva1": st8_pool.tile([128, 2 * NIL], f32,
                                              tag="mva1", name="mva1")}

            # Stage 0: scores(c0) interleaved with the projections so the
            # PE is never idle while exp(c0) streams on the scalar engine.
            cur = new_state(chunks[0])
            with (
                tc.tile_pool(name="pp_kt", bufs=1,
                             space=bass.MemorySpace.PSUM) as pp_kt,
                tc.tile_pool(name="pp_qq", bufs=1,
                             space=bass.MemorySpace.PSUM) as pp_qq,
                tc.tile_pool(name="pp_v", bufs=2,
                             space=bass.MemorySpace.PSUM) as pp_v,
            ):
                nc.vector.memset(
                    v_sb[:].rearrange("p (n c) -> p n c", c=VST)
                    [:, :, HID:HID + 1], 1.0)
                for c, h in ((0, 0), (0, 1), (1, 0), (1, 1)):
                    emit_kt_half(pp_kt, c, h)
                emit_qq_chunk(pp_qq, 0)
                emit_qq_chunk(pp_qq, 1)
                filler = ([("kt", (2, 0)), ("kt", (2, 1)),
                           ("kt", (3, 0)), ("kt", (3, 1))]
                          + [("qq", i) for i in range(2, 8)]
                          + [("v", (s, jp)) for s in range(SEGS)
                             for jp in range(NJP)])
                fi = 0
                for k in range(NJP):
                    scores_beat(cur, k)
                    for _ in range(6):
                        if fi < len(filler):
                            kind, arg = filler[fi]
                            fi += 1
                            if kind == "kt":
                                emit_kt_half(pp_kt, *arg)
                            elif kind == "qq":
                                emit_qq_chunk(pp_qq, arg)
                            else:
                                emit_v_pair(pp_v, *arg)
                while fi < len(filler):
                    kind, arg = filler[fi]
                    fi += 1
                    if kind == "kt":
                        emit_kt_half(pp_kt, *arg)
                    elif kind == "qq":
                        emit_qq_chunk(pp_qq, arg)
                    else:
                        emit_v_pair(pp_v, *arg)
            prev = cur

            with (
                tc.tile_pool(name="ps_att", bufs=2,
                             space=bass.MemorySpace.PSUM) as ps_att,
                tc.tile_pool(name="ps_fc", bufs=1,
                             space=bass.MemorySpace.PSUM) as ps_fc,
                tc.tile_pool(name="ps_tp", bufs=1,
                             space=bass.MemorySpace.PSUM) as ps_tp,
            ):
                for c in chunks[1:]:
                    cur = new_state(c)
                    for k in range(NJP):
                        scores_beat(cur, k)
                        att_beat(ps_att, prev, k)
                        if k == 4:
                            finish_group(ps_fc, ps_tp, prev, 0)
                    finish_group(ps_fc, ps_tp, prev, 1)
                    prev = cur
                for k in range(NJP):
                    att_beat(ps_att, prev, k)
                    if k == 4:
                        finish_group(ps_fc, ps_tp, prev, 0)
                finish_group(ps_fc, ps_tp, prev, 1)

    nc.compile()
    return nc


def _get_nc(apply0: bool):
    key = (bool(apply0),)
    if key not in _built:
        _built[key] = _build(apply0)
    return _built[key]


def _shard(inputs, apply0):
    from concourse import mybir
    bf = mybir.dt.np(mybir.dt.bfloat16)
    f8np = mybir.dt.np(mybir.dt.float8e4)

    q = np.ascontiguousarray(np.asarray(inputs["q"], dtype=np.float32))
    h = np.ascontiguousarray(np.asarray(inputs["h"], dtype=np.float32))
    WQ = np.asarray(inputs["WQ"], dtype=np.float32)
    WK = np.asarray(inputs["WK"], dtype=np.float32)
    WV = np.asarray(inputs["WV"], dtype=np.float32)
    fcw = np.asarray(inputs["fc_w"], dtype=np.float32)
    fcb = np.asarray(inputs["fc_b"], dtype=np.float32)

    WQT = np.ascontiguousarray(WQ.T).astype(bf)
    WKT = np.ascontiguousarray(WK.T).astype(bf)
    WVT = np.ascontiguousarray(WV.T).astype(np.float32)
    WV8 = np.ascontiguousarray(
        WVT.reshape(2, 128, HID).transpose(1, 0, 2).reshape(128, 2 * HID)
    ).astype(f8np)
    FCWT = np.ascontiguousarray(fcw.T).astype(bf)
    FCB = np.ascontiguousarray(fcb.reshape(1, HID)).astype(bf)
    IDT = np.eye(128, dtype=np.float32).astype(bf)

    in_maps = []
    for c in range(NCORES):
        sl = slice(c * ROWS, (c + 1) * ROWS)
        hTc = np.ascontiguousarray(h[sl].T)
        HT8 = np.ascontiguousarray(
            hTc.reshape(2, 128, ROWS).transpose(1, 0, 2).reshape(128, -1)
        ).astype(f8np)
        m = {
            "qT": np.ascontiguousarray(q[sl].T).astype(bf),
            "q": q[sl],
            "hT": hTc.astype(bf),
            "WQT": WQT, "WKT": WKT, "WV8": WV8, "HT8": HT8,
            "FCWT": FCWT, "FCB": FCB, "IDT": IDT,
        }
        if apply0:
            m["N0W"] = np.ascontiguousarray(
                np.broadcast_to(np.asarray(inputs["norm0_w"], np.float32),
                                (128, HID)))
            m["N0B"] = np.ascontiguousarray(
                np.broadcast_to(np.asarray(inputs["norm0_b"], np.float32),
                                (128, HID)))
        in_maps.append(m)
    return in_maps


def _run(inputs, trace=False, tmpdir=None):
    from concourse import bass_utils

    n0w = np.asarray(inputs["norm0_w"], np.float32)
    n0b = np.asarray(inputs["norm0_b"], np.float32)
    n1w = np.asarray(inputs["norm1_w"], np.float32)
    n1b = np.asarray(inputs["norm1_b"], np.float32)
    apply0 = not (np.allclose(n0w, 1.0) and np.allclose(n0b, 0.0))
    apply1 = not (np.allclose(n1w, 1.0) and np.allclose(n1b, 0.0))

    nc = _get_nc(apply0)
    in_maps = _shard(inputs, apply0)
    res = bass_utils.run_bass_kernel_spmd(
        nc, in_maps, core_ids=list(range(NCORES)), trace=trace,
        tmpdir=tmpdir)
    out = np.concatenate([np.asarray(res.results[c]["out"])
                          for c in range(NCORES)], axis=0)
    if apply1:
        out = out * n1w[None, :] + n1b[None, :]
    return out.astype(np.float32), res


def kernel(**inputs):
    out, _ = _run(inputs, trace=False)
    return out


# revision 16
# speedup vs baseline: 1.8002x; 1.2234x over previous
"""Trainium2 Bass kernel for nn_AttentionBlock (ragged_sequence, 16 equal
segments of 2048 q/kv tokens, HID=256, QD=64) on 8 NeuronCores.

Sharding: 2 segments (4096 rows) per core, weights replicated, outputs
concatenated host-side (attention is block-diagonal per segment -> no
cross-core communication needed).

v2: software-pipelined scores/exp vs att/epilogue, fp8 P+V with DoubleRow
att matmuls, wide exp slices, gpsimd epilogue offload.
"""

import os
import sys

os.environ.setdefault("MYCRO_LOCAL_CACHE", "1")
if "/opt/trn_rl_repo" not in sys.path:
    sys.path.insert(0, "/opt/trn_rl_repo")

import numpy as np

HID = 256
QD = 64
LQ = 2048
LH = 2048
B = 16
NCORES = 8
SEGS = 2                  # segments per core
ROWS = SEGS * LQ          # 4096 q rows per core
EPS = 1e-5
SCALE = 1.0 / 8.0         # 1/sqrt(QD)
NJT = LH // 128           # 16 key tiles per segment
NJP = NJT // 2            # 8 key-tile pairs per segment
NIC = 2                   # 1024-col query chunks per segment
ICW = LQ // NIC           # 1024
NIL = ICW // 128          # 8 query row-tiles per chunk
VST = 272                 # fp8 V block stride (256 V + 1 ones + pad, 16-aligned)

_built = {}


def _patch_act_tables():
    """Make the act-table pass choose the combined exp+ln table for every
    activation: blank all other tables (indices preserved so walrus's
    act_func_set_id remap stays correct). Avoids 100+ ACT_TABLE_LOADs
    (1.28us each) from alternating Exp/Ln table picks."""
    import functools
    import concourse.hw_specs as hw_specs
    import concourse.bacc as bacc_mod
    if getattr(hw_specs, "_attn_tables_patched", False):
        return
    orig = hw_specs.get_activation_tables

    @functools.cache
    def patched(arch):
        tabs = dict(orig(arch))
        joint = "natural_log_exp_and_others"
        assert joint in tabs, sorted(tabs)
        return {name: (funcs if name == joint else set())
                for name, funcs in tabs.items()}

    hw_specs.get_activation_tables = patched
    bacc_mod.get_activation_tables = patched
    hw_specs._attn_tables_patched = True


def _build(apply0: bool):
    from concourse import bacc, bass, mybir, tile

    _patch_act_tables()

    dt = mybir.dt
    f32 = dt.float32
    bf16 = dt.bfloat16
    f8 = dt.float8e4
    AF = mybir.ActivationFunctionType
    Alu = mybir.AluOpType
    DR = mybir.MatmulPerfMode.DoubleRow

    nc = bacc.Bacc("TRN2", target_bir_lowering=False, debug=False,
                   enable_asserts=False)

    qT_d = nc.dram_tensor("qT", [HID, ROWS], bf16, kind="ExternalInput")
    q_d = nc.dram_tensor("q", [ROWS, HID], f32, kind="ExternalInput")
    hT_d = nc.dram_tensor("hT", [HID, ROWS], bf16, kind="ExternalInput")
    wqT_d = nc.dram_tensor("WQT", [HID, QD], bf16, kind="ExternalInput")
    wkT_d = nc.dram_tensor("WKT", [HID, QD], bf16, kind="ExternalInput")
    wv8_d = nc.dram_tensor("WV8", [128, 2 * HID], f8, kind="ExternalInput")
    hT8_d = nc.dram_tensor("HT8", [128, 2 * ROWS], f8, kind="ExternalInput")
    fwT_d = nc.dram_tensor("FCWT", [HID, HID], bf16, kind="ExternalInput")
    fb_d = nc.dram_tensor("FCB", [1, HID], bf16, kind="ExternalInput")
    idt_d = nc.dram_tensor("IDT", [128, 128], bf16, kind="ExternalInput")
    if apply0:
        n0w_d = nc.dram_tensor("N0W", [128, HID], f32, kind="ExternalInput")
        n0b_d = nc.dram_tensor("N0B", [128, HID], f32, kind="ExternalInput")
    out_d = nc.dram_tensor("out", [ROWS, HID], f32, kind="ExternalOutput")

    qT_a, q_a, hT_a = qT_d.ap(), q_d.ap(), hT_d.ap()
    out_a = out_d.ap()

    with tile.TileContext(nc) as tc:
        with (
            tc.tile_pool(name="const", bufs=1) as cpool,
            tc.tile_pool(name="kqq", bufs=1) as kqq_pool,
            tc.tile_pool(name="vsb", bufs=1) as v_pool,
            tc.tile_pool(name="qhT", bufs=1) as qh_pool,
            tc.tile_pool(name="pt", bufs=18) as pt_pool,
            tc.tile_pool(name="qrow", bufs=18) as q_pool,
            tc.tile_pool(name="ep", bufs=4) as ep_pool,
            tc.tile_pool(name="ep8", bufs=18) as ep8_pool,
            tc.tile_pool(name="st8", bufs=8) as st8_pool,
            tc.tile_pool(name="outp", bufs=6) as o_pool,
            tc.tile_pool(name="ps_st", bufs=1,
                         space=bass.MemorySpace.PSUM) as ps_st,
        ):
            # ---- constants ----
            wq_sb = cpool.tile([128, 2 * QD], bf16)     # [e, (chunk, c)]
            wk_sb = cpool.tile([128, 2 * QD], bf16)
            wv8_sb = cpool.tile([128, 2 * HID], f8)     # V weights, e-pairs
            fw_sb = cpool.tile([128, 2 * HID], bf16)    # fc_w.T chunks
            fb_sb = cpool.tile([1, HID], bf16)
            one_sb = cpool.tile([1, 128], bf16)
            idt_sb = cpool.tile([128, 128], bf16)
            for e in range(2):
                nc.sync.dma_start(wq_sb[:, e * QD:(e + 1) * QD],
                                  wqT_d.ap()[e * 128:(e + 1) * 128, :])
                nc.sync.dma_start(wk_sb[:, e * QD:(e + 1) * QD],
                                  wkT_d.ap()[e * 128:(e + 1) * 128, :])
                nc.sync.dma_start(fw_sb[:, e * HID:(e + 1) * HID],
                                  fwT_d.ap()[e * 128:(e + 1) * 128, :])
            nc.sync.dma_start(wv8_sb[:], wv8_d.ap()[:, :])
            nc.sync.dma_start(fb_sb[:], fb_d.ap()[:, :])
            nc.sync.dma_start(idt_sb[:], idt_d.ap()[:, :])
            nc.vector.memset(one_sb[:], 1.0)
            eps_sb = cpool.tile([128, 1], f32)
            nc.vector.memset(eps_sb[:], EPS)
            nb3_sb = cpool.tile([128, 1], f32)
            nc.vector.memset(nb3_sb[:], -3.0)
            if apply0:
                n0w_sb = cpool.tile([128, HID], f32)
                n0b_sb = cpool.tile([128, HID], f32)
                nc.sync.dma_start(n0w_sb[:], n0w_d.ap()[:, :])
                nc.sync.dma_start(n0b_sb[:], n0b_d.ap()[:, :])

            # persistent activations
            # kT2: row-tiled K^T. partitions 0-63: even key tiles, 64-127:
            # odd key tiles; per seg 8 pair-blocks of 128 cols.
            kT2_sb = kqq_pool.tile([128, SEGS * LH // 2], bf16)
            # qq2: qq^T duplicated into both partition halves.
            qq2_sb = kqq_pool.tile([128, ROWS], bf16)
            # V (fp8) row-layout: per (seg, jt) a 272-col block
            # (256 V + ones col at 256 + pad).
            v_sb = v_pool.tile([128, SEGS * NJT * VST], f8)

            # fp8 e-paired h^T for the V projection (DoubleRow)
            ht8_sb = v_pool.tile([128, 2 * ROWS], f8)

            # load qT / hT as 8 tiles each of [128, 1024], split into
            # 512-col DMA pieces spread over the queues. Priority: hT
            # (gates kT -> first scores), then qT chunk 0 (gates qq),
            # then the rest, then ht8 (V projection).
            qts = {}
            hts = {}
            for c in range(ROWS // 1024):
                for e in range(2):
                    t2 = qh_pool.tile([128, 1024], bf16, tag=f"ht{e}_{c}")
                    hts[(e, c)] = t2
                    t = qh_pool.tile([128, 1024], bf16, tag=f"qt{e}_{c}")
                    qts[(e, c)] = t

            def _load2(dst, src_ap, e, c):
                for h in range(2):
                    nc.sync.dma_start(
                        dst[:, h * 512:(h + 1) * 512],
                        src_ap[e * 128:(e + 1) * 128,
                               c * 1024 + h * 512:c * 1024 + (h + 1) * 512])

            for c in range(ROWS // 1024):
                for e in range(2):
                    _load2(hts[(e, c)], hT_a, e, c)
                if c == 0:
                    for e in range(2):
                        _load2(qts[(e, 0)], qT_a, e, 0)
            for c in range(1, ROWS // 1024):
                for e in range(2):
                    _load2(qts[(e, c)], qT_a, e, c)
            for c in range(8):
                nc.sync.dma_start(ht8_sb[:, c * 1024:(c + 1) * 1024],
                                  hT8_d.ap()[:, c * 1024:(c + 1) * 1024])

            def _slice(tiles, e, col, width):
                c, off = col // 1024, col % 1024
                assert off + width <= 1024
                return tiles[(e, c)][:, off:off + width]

            # ---------------- pipelined main loop ----------------
            chunks = [(s, ic) for s in range(SEGS) for ic in range(NIC)]

            def emit_kt_half(pp_kt, c, half):
                # hT cols [c*1024+half*512, +512) = key tiles 8c+4h..+3
                ps = pp_kt.tile([64, 512], f32, tag="kt")
                col = c * 1024 + half * 512
                for e in range(2):
                    nc.tensor.matmul(
                        ps[:],
                        wk_sb[:, e * QD:(e + 1) * QD],
                        _slice(hts, e, col, 512),
                        start=(e == 0), stop=(e == 1))
                # evac: even tiles -> partitions 0-63, odd -> 64-127
                src = ps[:].rearrange("p (n c) -> p n c", c=256)
                base = c * 512 + half * 256
                for par in range(2):
                    dst = kT2_sb[64 * par:64 * (par + 1), base:base + 256]
                    nc.vector.tensor_copy(
                        dst.rearrange("p (n c) -> p n c", c=128),
                        src[:, :, 128 * par:128 * (par + 1)])

            def emit_qq_chunk(pp_qq, c512):
                # qT cols [c512*512, +512), duplicated via column tiling
                ps = pp_qq.tile([128, 512], f32, tag="qq")
                col = c512 * 512
                for e in range(2):
                    nc.tensor.matmul(
                        ps[0:64, :],
                        wq_sb[:, e * QD:(e + 1) * QD],
                        _slice(qts, e, col, 512),
                        start=(e == 0), stop=(e == 1))
                for e in range(2):
                    nc.tensor.matmul(
                        ps[64:128, :],
                        wq_sb[:, e * QD:(e + 1) * QD],
                        _slice(qts, e, col, 512),
                        start=(e == 0), stop=(e == 1),
                        tile_position=(0, 64))
                nc.vector.tensor_copy(qq2_sb[:, col:col + 512], ps[:])

            def emit_v_pair(pp_v, s, jp):
                # two key tiles' V rows via one DoubleRow matmul each
                ps = pp_v.tile([128, 2 * HID], f32, tag="v")
                for u in range(2):
                    col = s * LH + (2 * jp + u) * 128
                    lhsT = (ht8_sb[:]
                            .rearrange("p (two r) -> p two r", two=2)
                            [:, :, col:col + 128])
                    rhs = (wv8_sb[:]
                           .rearrange("p (two d) -> p two d", two=2))
                    nc.tensor.matmul(ps[:, u * HID:(u + 1) * HID],
                                     lhsT, rhs, start=True, stop=True,
                                     perf_mode=DR)
                vb = 2 * (s * NJP + jp)
                dst = (v_sb[:].rearrange("p (n c) -> p n c", c=VST)
                       [:, vb:vb + 2, 0:HID])
                nc.vector.tensor_copy(
                    dst, ps[:].rearrange("p (n c) -> p n c", c=HID))

            def scores_beat(state, k):
                s, ic = state["c"]
                icol = s * LQ + ic * ICW
                if k == 0:
                    state["st"] = ps_st.tile([128, 2048], f32, tag="st",
                                             name="st")
                st = state["st"]
                kcol = s * (LH // 2) + k * 128
                for h in range(2):
                    nc.tensor.matmul(
                        st[:, h * 512:(h + 1) * 512],
                        kT2_sb[0:64, kcol:kcol + 128],
                        qq2_sb[0:64, icol + h * 512:icol + (h + 1) * 512],
                        start=True, stop=True)
                    nc.tensor.matmul(
                        st[:, 1024 + h * 512:1024 + (h + 1) * 512],
                        kT2_sb[64:128, kcol:kcol + 128],
                        qq2_sb[64:128,
                               icol + h * 512:icol + (h + 1) * 512],
                        start=True, stop=True)
                pt2 = pt_pool.tile([128, 2048], f8, tag="pt")
                nc.scalar.activation(pt2[:, 0:1024], st[:, 0:1024], AF.Exp,
                                     scale=SCALE, bias=nb3_sb[:])
                nc.scalar.activation(pt2[:, 1024:2048], st[:, 1024:2048],
                                     AF.Exp, scale=SCALE, bias=nb3_sb[:])
                state["pts"].append(pt2)
                # prefetch q rows for this chunk's epilogue
                row0 = icol + k * 128
                qt = q_pool.tile([128, HID], f32, tag="q")
                nc.sync.dma_start(qt[:], q_a[row0:row0 + 128, :])
                state["qts"].append(qt)

            def att_beat(ps_att, state, il):
                s, ic = state["c"]
                att = ps_att.tile([128, 512], f32, tag="att")
                for jp in range(NJP):
                    lhsT = (state["pts"][jp][:]
                            .rearrange("p (two q) -> p two q", two=2)
                            [:, :, il * 128:(il + 1) * 128])
                    vb = 2 * (s * NJP + jp)
                    rhs = (v_sb[:]
                           .rearrange("p (n c) -> p n c", c=VST)
                           [:, vb:vb + 2, 0:HID + 1])
                    nc.tensor.matmul(att[:, 0:HID + 1], lhsT, rhs,
                                     start=(jp == 0), stop=(jp == NJP - 1),
                                     perf_mode=DR)
                qt = state["qts"][il]
                rden = st8_pool.tile([128, 1], f32, tag="rd")
                nc.vector.reciprocal(rden[:], att[:, HID:HID + 1])
                x0 = state["xb"][:, il * HID:(il + 1) * HID]
                nc.vector.scalar_tensor_tensor(
                    x0, att[:, 0:HID], rden[:].opt(), qt[:],
                    op0=Alu.mult, op1=Alu.add)

            def finish_group(ps_fc, ps_tp, state, g, gsize=4):
                s, ic = state["c"]
                # LN0 stats for this group of row-tiles
                mva0 = state["mva0"]
                mv24 = st8_pool.tile([128, 4 * 6], f32, tag="mv24")
                for il4 in range(gsize):
                    il = g * gsize + il4
                    nc.vector.bn_stats(
                        mv24[:, 6 * il4:6 * il4 + 6],
                        state["xb"][:, il * HID:(il + 1) * HID])
                    nc.vector.bn_aggr(mva0[:, 2 * il:2 * il + 2],
                                      mv24[:, 6 * il4:6 * il4 + 6])
                ln4a = st8_pool.tile([128, 4], f32, tag="ln4a")
                nc.scalar.activation(
                    ln4a[:, 0:gsize].rearrange("p (t o) -> p t o", o=1),
                    mva0[:, 2 * g * gsize:2 * (g + 1) * gsize]
                    .rearrange("p (t o) -> p t o", o=2)[:, :, 1:2],
                    AF.Ln, bias=eps_sb[:])
                rstd4a = st8_pool.tile([128, 4], f32, tag="r4a")
                nc.scalar.activation(rstd4a[:, 0:gsize], ln4a[:, 0:gsize],
                                     AF.Exp, scale=-0.5)

                zfs = []
                for il4 in range(gsize):
                    il = g * gsize + il4
                    x0 = state["xb"][:, il * HID:(il + 1) * HID]
                    z = ep_pool.tile([128, HID], bf16, tag="z")
                    nc.vector.tensor_scalar(
                        z[:], x0, mva0[:, 2 * il:2 * il + 1].opt(),
                        rstd4a[:, il4:il4 + 1].opt(),
                        op0=Alu.subtract, op1=Alu.mult)
                    if apply0:
                        z2 = ep_pool.tile([128, HID], bf16, tag="z2")
                        nc.gpsimd.tensor_tensor(z2[:], z[:], n0w_sb[:],
                                                op=Alu.mult)
                        z3 = ep_pool.tile([128, HID], bf16, tag="z3")
                        nc.gpsimd.tensor_tensor(z3[:], z2[:], n0b_sb[:],
                                                op=Alu.add)
                        zfs.append(z3)
                    else:
                        zfs.append(z)
                # batched transposes (one PE mode switch per group)
                tp = ps_tp.tile([128, 1024], bf16, tag="tp")
                for il4 in range(gsize):
                    for hh in range(2):
                        nc.tensor.transpose(
                            tp[:, il4 * 256 + hh * 128:
                               il4 * 256 + (hh + 1) * 128],
                            zfs[il4][:, hh * 128:(hh + 1) * 128],
                            idt_sb[:])
                zT = ep_pool.tile([128, 1024], bf16, tag="zT")
                nc.vector.tensor_copy(zT[:, 0:gsize * 256],
                                      tp[:, 0:gsize * 256])
                for il4 in range(gsize):
                    il = g * gsize + il4
                    hres = ps_fc.tile([128, HID], f32, tag="fc")
                    nc.tensor.matmul(hres[:], one_sb[:], fb_sb[:],
                                     start=True, stop=False)
                    for hh in range(2):
                        nc.tensor.matmul(
                            hres[:],
                            zT[:, il4 * 256 + hh * 128:
                               il4 * 256 + (hh + 1) * 128],
                            fw_sb[:, hh * HID:(hh + 1) * HID],
                            start=False, stop=(hh == 1))
                    y0 = state["yb"][:, il * HID:(il + 1) * HID]
                    nc.vector.scalar_tensor_tensor(
                        y0, hres[:], 0.0, zfs[il4][:],
                        op0=Alu.max, op1=Alu.add)

                mva1 = state["mva1"]
                mv24b = st8_pool.tile([128, 4 * 6], f32, tag="mv24b")
                for il4 in range(gsize):
                    il = g * gsize + il4
                    nc.vector.bn_stats(
                        mv24b[:, 6 * il4:6 * il4 + 6],
                        state["yb"][:, il * HID:(il + 1) * HID])
                    nc.vector.bn_aggr(mva1[:, 2 * il:2 * il + 2],
                                      mv24b[:, 6 * il4:6 * il4 + 6])
                ln4b = st8_pool.tile([128, 4], f32, tag="ln4b")
                nc.scalar.activation(
                    ln4b[:, 0:gsize].rearrange("p (t o) -> p t o", o=1),
                    mva1[:, 2 * g * gsize:2 * (g + 1) * gsize]
                    .rearrange("p (t o) -> p t o", o=2)[:, :, 1:2],
                    AF.Ln, bias=eps_sb[:])
                rstd4b = st8_pool.tile([128, 4], f32, tag="r4b")
                nc.scalar.activation(rstd4b[:, 0:gsize], ln4b[:, 0:gsize],
                                     AF.Exp, scale=-0.5)

                icol = s * LQ + ic * ICW
                for il4 in range(gsize):
                    il = g * gsize + il4
                    row0 = icol + il * 128
                    ot = o_pool.tile([128, HID], f32, tag="ot")
                    nc.vector.tensor_scalar(
                        ot[:], state["yb"][:, il * HID:(il + 1) * HID],
                        mva1[:, 2 * il:2 * il + 1].opt(),
                        rstd4b[:, il4:il4 + 1].opt(),
                        op0=Alu.subtract, op1=Alu.mult)
                    nc.sync.dma_start(out_a[row0:row0 + 128, :], ot[:])

            def new_state(c):
                return {"c": c, "pts": [], "qts": [],
                        "xb": ep8_pool.tile([128, NIL * HID], bf16,
                                            tag="xb", name="xb", bufs=3),
                        "yb": ep8_pool.tile([128, NIL * HID], bf16,
                                            tag="yb", name="yb", bufs=3),
                        "mva0": st8_pool.tile([128, 2 * NIL], f32,
                                              tag="mva0", name="mva0"),
                        "mva1": st8_pool.tile([128, 2 * NIL], f32,
                                              tag="mva1", name="mva1")}

            # Stage 0: scores(c0) interleaved with the projections so the
            # PE is never idle while exp(c0) streams on the scalar engine.
            cur = new_state(chunks[0])
            with (
                tc.tile_pool(name="pp_kt", bufs=1,
                             space=bass.MemorySpace.PSUM) as pp_kt,
                tc.tile_pool(name="pp_qq", bufs=1,
                             space=bass.MemorySpace.PSUM) as pp_qq,
                tc.tile_pool(name="pp_v", bufs=2,
                             space=bass.MemorySpace.PSUM) as pp_v,
            ):
                nc.vector.memset(
                    v_sb[:].rearrange("p (n c) -> p n c", c=VST)
                    [:, :, HID:HID + 1], 1.0)
                for c, h in ((0, 0), (0, 1), (1, 0), (1, 1)):
                    emit_kt_half(pp_kt, c, h)
                emit_qq_chunk(pp_qq, 0)
                emit_qq_chunk(pp_qq, 1)
                filler = ([("kt", (2, 0)), ("kt", (2, 1)),
                           ("kt", (3, 0)), ("kt", (3, 1))]
                          + [("qq", i) for i in range(2, 8)]
                          + [("v", (s, jp)) for s in range(SEGS)
                             for jp in range(NJP)])
                fi = 0
                for k in range(NJP):
                    scores_beat(cur, k)
                    for _ in range(6):
                        if fi < len(filler):
                            kind, arg = filler[fi]
                            fi += 1
                            if kind == "kt":
                                emit_kt_half(pp_kt, *arg)
                            elif kind == "qq":
                                emit_qq_chunk(pp_qq, arg)
                            else:
                                emit_v_pair(pp_v, *arg)
                while fi < len(filler):
                    kind, arg = filler[fi]
                    fi += 1
                    if kind == "kt":
                        emit_kt_half(pp_kt, *arg)
                    elif kind == "qq":
                        emit_qq_chunk(pp_qq, arg)
                    else:
                        emit_v_pair(pp_v, *arg)
            prev = cur

            with (
                tc.tile_pool(name="ps_att", bufs=2,
                             space=bass.MemorySpace.PSUM) as ps_att,
                tc.tile_pool(name="ps_fc", bufs=1,
                             space=bass.MemorySpace.PSUM) as ps_fc,
                tc.tile_pool(name="ps_tp", bufs=1,
                             space=bass.MemorySpace.PSUM) as ps_tp,
            ):
                pprev = None
                for c in chunks[1:]:
                    cur = new_state(c)
                    for k in range(NJP):
                        scores_beat(cur, k)
                        att_beat(ps_att, prev, k)
                        if k == 1 and pprev is not None:
                            finish_group(ps_fc, ps_tp, pprev, 1)
                        if k == 5:
                            finish_group(ps_fc, ps_tp, prev, 0)
                    pprev = prev
                    prev = cur
                # drain: finer finish groups to cut the tail latency
                finish_group(ps_fc, ps_tp, pprev, 1)
                for k in range(NJP):
                    att_beat(ps_att, prev, k)
                    if k >= 3 and k % 2 == 1:
                        finish_group(ps_fc, ps_tp, prev, (k - 3) // 2,
                                     gsize=2)
                finish_group(ps_fc, ps_tp, prev, 3, gsize=2)

    nc.compile()
    return nc


def _get_nc(apply0: bool):
    key = (bool(apply0),)
    if key not in _built:
        _built[key] = _build(apply0)
    return _built[key]


def _shard(inputs, apply0):
    from concourse import mybir
    bf = mybir.dt.np(mybir.dt.bfloat16)
    f8np = mybir.dt.np(mybir.dt.float8e4)

    q = np.ascontiguousarray(np.asarray(inputs["q"], dtype=np.float32))
    h = np.ascontiguousarray(np.asarray(inputs["h"], dtype=np.float32))
    WQ = np.asarray(inputs["WQ"], dtype=np.float32)
    WK = np.asarray(inputs["WK"], dtype=np.float32)
    WV = np.asarray(inputs["WV"], dtype=np.float32)
    fcw = np.asarray(inputs["fc_w"], dtype=np.float32)
    fcb = np.asarray(inputs["fc_b"], dtype=np.float32)

    WQT = np.ascontiguousarray(WQ.T).astype(bf)
    WKT = np.ascontiguousarray(WK.T).astype(bf)
    WVT = np.ascontiguousarray(WV.T).astype(np.float32)
    WV8 = np.ascontiguousarray(
        WVT.reshape(2, 128, HID).transpose(1, 0, 2).reshape(128, 2 * HID)
    ).astype(f8np)
    FCWT = np.ascontiguousarray(fcw.T).astype(bf)
    FCB = np.ascontiguousarray(fcb.reshape(1, HID)).astype(bf)
    IDT = np.eye(128, dtype=np.float32).astype(bf)

    in_maps = []
    for c in range(NCORES):
        sl = slice(c * ROWS, (c + 1) * ROWS)
        hTc = np.ascontiguousarray(h[sl].T)
        HT8 = np.ascontiguousarray(
            hTc.reshape(2, 128, ROWS).transpose(1, 0, 2).reshape(128, -1)
        ).astype(f8np)
        m = {
            "qT": np.ascontiguousarray(q[sl].T).astype(bf),
            "q": q[sl],
            "hT": hTc.astype(bf),
            "WQT": WQT, "WKT": WKT, "WV8": WV8, "HT8": HT8,
            "FCWT": FCWT, "FCB": FCB, "IDT": IDT,
        }
        if apply0:
            m["N0W"] = np.ascontiguousarray(
                np.broadcast_to(np.asarray(inputs["norm0_w"], np.float32),
                                (128, HID)))
            m["N0B"] = np.ascontiguousarray(
                np.broadcast_to(np.asarray(inputs["norm0_b"], np.float32),
                                (128, HID)))
        in_maps.append(m)
    return in_maps


def _run(inputs, trace=False, tmpdir=None):
    from concourse import bass_utils

    n0w = np.asarray(inputs["norm0_w"], np.float32)
    n0b = np.asarray(inputs["norm0_b"], np.float32)
    n1w = np.asarray(inputs["norm1_w"], np.float32)
    n1b = np.asarray(inputs["norm1_b"], np.float32)
    apply0 = not (np.allclose(n0w, 1.0) and np.allclose(n0b, 0.0))
    apply1 = not (np.allclose(n1w, 1.0) and np.allclose(n1b, 0.0))

    nc = _get_nc(apply0)
    in_maps = _shard(inputs, apply0)
    res = bass_utils.run_bass_kernel_spmd(
        nc, in_maps, core_ids=list(range(NCORES)), trace=trace,
        tmpdir=tmpdir)
    out = np.concatenate([np.asarray(res.results[c]["out"])
                          for c in range(NCORES)], axis=0)
    if apply1:
        out = out * n1w[None, :] + n1b[None, :]
    return out.astype(np.float32), res


def kernel(**inputs):
    out, _ = _run(inputs, trace=False)
    return out
